# revision 43
# baseline (speedup 1.0000x reference)
"""Conv2d 3x3 same-padding, NCHW, on 8 TRN2 NeuronCores (data-parallel).

Problem: x[32,128,56,56] f32, weight[256,128,3,3] OIHW, bias[256] ->
y[32,256,56,56].  Batch is sharded 4 images/core; weight+bias replicated.

Per-core Winograd F(2,3) along W (bf16 matmuls, fp32 PSUM):
  - W=56 output cols -> 28 tiles of 2.  For tile j the 4 input taps are
    padded cols 2j..2j+3.  Input transform (W only, per row):
      v0 = d0-d2, v1 = d1+d2, v2 = d2-d1, v3 = d1-d3
    all computed on DVE (fastest engine for bf16 tensor_tensor), one
    image ahead, as 8 half-image sub-ops spread over the groups.
  - weights are host-transformed: U[xi][co,ci,kh] = sum_kw G[xi,kw] w[..kw],
    G = [[1,0,0],[.5,.5,.5],[.5,-.5,.5],[0,0,1]], laid out as
    [ci, ch, xi, kh, co128] bf16 so each (ch,xi,kh) is a [128,128] lhsT.
  - PE per (img, ch-half, 28-row GROUP = 2 chunks of 14 rows): per xi one
    PSUM tile [128, 2, 512] f32 (2 banks); chunk c -> ps[:, c, 0:392].
    24 matmuls of 392 cols per group; all 8 PSUM banks in use; group
    pipelining relies on per-xi-tile WAR (evictions free banks in xi
    order 1,2,0,3 matching the matmul order).
  - combine per group (y0 = m0+m1+m2+b, y1 = m1-m2-m3+b) with wide
    [2,392] strided PSUM reads, engine-balanced by measured rates
    (ACT (172+FD)/1.2, DVE (120+FD)/0.96 psum / (58+FD/2)/0.96 bf16,
    GPS ~2.1ns/elem, and GPS shares its SBUF port with DVE so it gets
    exactly one op per group):
      ACT: e1 = Id(m1+b), e2 = Copy(m2)            (bf16 out)
      DVE: q = m0+e1, y0 = q+e2, y1 = d-m3         -> ot bf16
      GPS: d = e1-e2
    q = m0+e1 frees m0's banks with only the e1 dependency, keeping the
    next-group WAR chain short (~3us slack).
  - DMA: all input DMAs go on the sync queue, whose per-queue emission
    order is the ring service order (the 16 DMA engines round-robin
    BETWEEN queues, so a big transfer on another queue would starve the
    startup-critical x0/u0 transfers; the Tile scheduler also hoists
    dependency-free DMA issues to t~0 regardless of program position).
    Priority order: x0 rows 0..15, u[ch0] first half (xi 1,2 in matmul
    order thanks to the host-side xi permute), x0 rows 16..29, u[ch0]
    second half, then the rest after the first group's matmuls are
    enqueued; xp for image i+1 is issued a full image ahead.
  - output is bf16 parity-split [128, 2, 4, 392]; host re-interleaves W
    and casts to f32.  One DMA per (img, ch-half); the last img-ch is
    drained per chunk so only ~1 chunk's combine+DMA trails the last mm.
  - image 0 x is DMA'd in 4 disjoint row-ranges so the PE starts after
    ~230 KB lands; warm-up matmuls trip the PE HAM clock-gate during the
    startup window.
"""

import ml_dtypes
import numpy as np

import concourse.bacc as bacc
import concourse.mybir as mybir
import concourse.tile as tile
from concourse.bass_utils import run_bass_kernel_spmd

N_CORES = 8
N, C_IN, H, W = 32, 128, 56, 56
C_OUT, KH, KW = 256, 3, 3
PER = N // N_CORES          # images per core
HP, WP = H + 2, W + 2       # zero-padded image dims
NT = W // 2                 # 28 winograd tiles along W
NXI = 4                     # winograd terms
RPC = 14                    # output rows per chunk
FD = RPC * NT               # 392 matmul cols per chunk
N_GRP = 2                   # 2-chunk groups per img-ch (28 rows each)
N_CH = C_OUT // 128         # output-channel halves
WARMUP_MMS = 3
# matmul group order (evictions start after m1, m2); the host stores U's
# xi axis permuted into this order so u[ch0] splits into two contiguous
# priority DMAs (first half = xi 1,2 gates the first matmuls)
XI_ORDER = (1, 2, 0, 3)
# image-0 row ranges (disjoint, cover 0..57)
R0 = [(0, 16), (16, 30), (30, 44), (44, 58)]
# half-image row splits for the look-ahead V transform
VR = [(0, 29), (29, 58)]

f32 = mybir.dt.float32
bf16 = mybir.dt.bfloat16
ADD = mybir.AluOpType.add
SUB = mybir.AluOpType.subtract
COPY = mybir.ActivationFunctionType.Copy
IDENT = mybir.ActivationFunctionType.Identity

_prog_cache = {}


def _build_program():
    nc = bacc.Bacc("TRN2", target_bir_lowering=False, debug=False)
    # x is host-split into even/odd column planes [.., HP, 2, 29] and y is
    # produced parity-split bf16 [.., 2, 4, 392] (host re-interleaves after
    # the run) so every vector-engine access on chip is contiguous.
    x_d = nc.declare_dram_parameter("x", [PER, C_IN, HP, 2, 29], bf16, isOutput=False)
    u_d = nc.declare_dram_parameter("u", [C_IN, N_CH, NXI, KH, 128], bf16, isOutput=False)
    b_d = nc.declare_dram_parameter("b", [128, N_CH], f32, isOutput=False)
    y_d = nc.declare_dram_parameter("y", [PER, N_CH, 128, 2, 4, FD], bf16, isOutput=True)

    with tile.TileContext(nc) as tc:
        with (
            tc.tile_pool(name="wpool", bufs=1) as wpool,
            tc.tile_pool(name="x0pool", bufs=4) as x0pool,
            tc.tile_pool(name="xppool", bufs=2) as xppool,
            tc.tile_pool(name="vpool", bufs=2) as vpool,
            tc.tile_pool(name="spool", bufs=10) as spool,
            tc.tile_pool(name="opool", bufs=3) as opool,
            tc.tile_pool(name="pspool", bufs=4, space="PSUM") as pspool,
            tc.tile_pool(name="warm", bufs=1) as warmpool,
        ):
            # PE warm-up during the startup protocol / first DMA window.
            wu_zero = warmpool.tile([128, FD], f32, tag="wuzero")
            nc.vector.memset(wu_zero[:], 0.0)
            wu_src = warmpool.tile([128, FD], bf16, tag="wusrc")
            nc.vector.tensor_copy(wu_src[:], wu_zero[:])
            wu_ps = pspool.tile([128, 2, 512], f32, tag="ps")

            u_t = wpool.tile([C_IN, N_CH, NXI, KH, 128], bf16, tag="u")
            b_t = wpool.tile([128, N_CH], f32, tag="b")

            # image-0 row-range tiles + shared V tile
            x0c = [x0pool.tile([128, 16, 2, 29], bf16, tag="x0", name=f"x0c{k}")
                   for k in range(len(R0))]
            v0 = vpool.tile([128, NXI, HP, NT], bf16, tag="v")

            # Input DMAs all stripe across the same 16 DMA engines, and
            # emission order on one queue (sync) is the ring's service
            # order.  So: only what the FIRST matmuls need goes first --
            # u[ch0,xi=1], x0 rows 0..29 -- then the rest of u[ch0] in
            # matmul (xi) order; everything else after the first group's
            # matmuls are enqueued.
            nc.sync.dma_start(x0c[0][:, 0:R0[0][1] - R0[0][0]], x_d[0, :, R0[0][0]:R0[0][1]])
            nc.sync.dma_start(u_t[:, 0, 0:2], u_d[:, 0, 0:2])
            nc.sync.dma_start(x0c[1][:, 0:R0[1][1] - R0[1][0]], x_d[0, :, R0[1][0]:R0[1][1]])
            nc.sync.dma_start(u_t[:, 0, 2:4], u_d[:, 0, 2:4])
            nc.scalar.dma_start(b_t[:], b_d[:])

            for _ in range(WARMUP_MMS):
                nc.tensor.matmul(wu_ps[:, 0, 0:FD], wu_src[:, :128], wu_src[:],
                                 start=True, stop=True)

            def v_transform(eng, vt, xt, rows_out, rows_in, only_xi=None):
                """vt[:, xi, rows_out, :] from xt rows rows_in (same rows).
                With d_t = xpad[2j+t]: v0=d0-d2, v1=d1+d2, v2=d2-d1,
                v3=d1-d3.  x is even/odd split: d0=xe[j], d2=xe[j+1],
                d1=xo[j], d3=xo[j+1] -- all contiguous 28-slices."""
                o0, o1 = rows_out
                i0, i1 = rows_in
                xe0 = xt[:, i0:i1, 0, 0:NT]
                xe1 = xt[:, i0:i1, 0, 1:NT + 1]
                xo0 = xt[:, i0:i1, 1, 0:NT]
                xo1 = xt[:, i0:i1, 1, 1:NT + 1]
                table = [
                    (0, xe0, xe1, SUB),
                    (1, xo0, xe1, ADD),
                    (2, xe1, xo0, SUB),
                    (3, xo0, xo1, SUB),
                ]
                for xi, s0, s1, op in table:
                    if only_xi is None or xi == only_xi:
                        eng.tensor_tensor(vt[:, xi, o0:o1, :], s0, s1, op)

            # image-0 V for rows 0..29 on DVE right behind the DMAs, xi-major
            # in matmul order so the PE's next-needed plane is always first
            for xi in XI_ORDER:
                for k in (0, 1):
                    v_transform(nc.vector, v0, x0c[k], R0[k],
                                (0, R0[k][1] - R0[k][0]), only_xi=xi)

            vts = {0: v0}
            xps = [xppool.tile([128, HP, 2, 29], bf16, tag="xp",
                               name=f"xp{i}") for i in range(PER - 1)]

            for img in range(PER):
                vt = vts.pop(img)
                if img + 1 < PER:
                    xp = xps[img]
                    vnxt = vpool.tile([128, NXI, HP, NT], bf16, tag="v")
                    vts[img + 1] = vnxt
                # look-ahead V sub-ops: (xi, row-range).  The xp DMA for
                # img+1 is issued a full image ahead (except img 1's, which
                # waits for the startup backlog), so by the time these run
                # the data is resident.  img 0 skips group 0 because xp1
                # lands only mid-image.
                vops = [(xi, vr) for vr in VR for xi in range(NXI)]
                vops_per_group = ({1: 3, 2: 3, 3: 2} if img == 0
                                  else {0: 2, 1: 2, 2: 2, 3: 2})
                vop_i = 0
                for ch in range(N_CH):
                    ot = opool.tile([128, 2, 2 * N_GRP, FD], bf16, tag="ot")
                    bias = b_t[:, ch:ch + 1]
                    for g in range(N_GRP):
                        last_grp = (img == PER - 1 and ch == N_CH - 1)
                        ps = {}
                        for kpos, xi in enumerate(XI_ORDER):
                            pst = pspool.tile([128, 2, 512], f32, tag="ps")
                            ps[xi] = pst
                            for kh in range(KH):
                                for c in range(2):
                                    r0 = RPC * (2 * g + c) + kh
                                    nc.tensor.matmul(
                                        pst[:, c, 0:FD],
                                        u_t[:, ch, kpos, kh, :],
                                        vt[:, xi, r0:r0 + RPC, :],
                                        start=(kh == 0),
                                        stop=(kh == KH - 1),
                                    )
                        if img == 0 and ch == 0 and g == 0:
                            # deferred input DMAs, still priority-ordered on
                            # the sync queue behind u0/x0c0/x0c1
                            nc.sync.dma_start(x0c[2][:, 0:R0[2][1] - R0[2][0]],
                                              x_d[0, :, R0[2][0]:R0[2][1]])
                            nc.sync.dma_start(x0c[3][:, 0:R0[3][1] - R0[3][0]],
                                              x_d[0, :, R0[3][0]:R0[3][1]])
                            nc.sync.dma_start(u_t[:, 1], u_d[:, 1])
                            for xi in XI_ORDER:
                                for k in (2, 3):
                                    v_transform(nc.vector, v0, x0c[k], R0[k],
                                                (0, R0[k][1] - R0[k][0]),
                                                only_xi=xi)
                        if img == 0 and ch == 0 and g == 1 and PER > 1:
                            nc.sync.dma_start(xps[0][:], x_d[1])
                        if ch == 1 and g == 0 and img + 2 < PER:
                            nc.sync.dma_start(xps[img + 1][:], x_d[img + 2])
                        # look-ahead V sub-ops on DVE, FIRST in the group's
                        # DVE FIFO slot: they have no unmet deps, so they run
                        # in the PE's m1/m2 window while DVE is idle, and the
                        # image's full V is ready before the next image's
                        # first matmul.
                        if img + 1 < PER:
                            for vk in range(vops_per_group.get(2 * ch + g, 0)):
                                xi, (a, b) = vops[vop_i]
                                vop_i += 1
                                xe0 = xp[:, a:b, 0, 0:NT]
                                xe1 = xp[:, a:b, 0, 1:NT + 1]
                                xo0 = xp[:, a:b, 1, 0:NT]
                                xo1 = xp[:, a:b, 1, 1:NT + 1]
                                src0, src1, op = [
                                    (xe0, xe1, SUB),   # v0 = d0-d2
                                    (xo0, xe1, ADD),   # v1 = d1+d2
                                    (xe1, xo0, SUB),   # v2 = d2-d1
                                    (xo0, xo1, SUB),   # v3 = d1-d3
                                ][xi]
                                # one V sub-op per group rides on GPSIMD
                                # (freed by d moving to DVE); it feeds the
                                # NEXT image, so its latency is harmless
                                eng = nc.gpsimd if vk == 0 else nc.vector
                                eng.tensor_tensor(
                                    vts[img + 1][:, xi, a:b, :], src0, src1, op)
                        # combine: y0 = m0+m1+m2+b, y1 = m1-m2-m3+b over
                        # both chunks at once ([2,392] strided PSUM reads).
                        # q = m0+e1 on DVE frees m0's banks with only the
                        # e1 dependency (not e2), so the next-group WAR
                        # chain is short; d on GPSIMD (its consumer y1 has
                        # slack, and one op per group keeps DVE/GPS
                        # SBUF-port contention low).
                        m0, m1, m2, m3 = (ps[i][:, :, 0:FD] for i in range(4))
                        e1 = spool.tile([128, 2, FD], bf16, tag="e1")
                        e2 = spool.tile([128, 2, FD], bf16, tag="e2")
                        qt = spool.tile([128, 2, FD], bf16, tag="q")
                        dt = spool.tile([128, 2, FD], bf16, tag="d")
                        nc.scalar.activation(e1[:], m1, IDENT, bias=bias)
                        nc.scalar.activation(e2[:], m2, COPY)
                        nc.vector.tensor_tensor(qt[:], m0, e1[:], ADD)
                        nc.vector.tensor_tensor(
                            ot[:, 0, 2 * g:2 * g + 2, :], qt[:], e2[:], ADD)
                        if last_grp and g == N_GRP - 1:
                            # final drain: per-chunk y-ops and DMAs on DVE so
                            # only ~1 chunk's combine+DMA trails the last mm
                            nc.vector.tensor_tensor(dt[:], e1[:], e2[:], SUB)
                            for c in range(2):
                                nc.vector.tensor_tensor(
                                    ot[:, 1, 2 * g + c, :], dt[:, c],
                                    ps[3][:, c, 0:FD], SUB)
                                nc.sync.dma_start(
                                    y_d[img, ch, :, :, 2 * g + c, :],
                                    ot[:, :, 2 * g + c, :])
                        else:
                            # d on DVE (bf16, 469ns) so y1 -- the m3 tile's
                            # WAR-critical last reader -- fires ~0.6us
                            # earlier than via the 1.7us GPSIMD d
                            nc.vector.tensor_tensor(dt[:], e1[:], e2[:], SUB)
                            nc.vector.tensor_tensor(
                                ot[:, 1, 2 * g:2 * g + 2, :], dt[:], m3, SUB)
                            if last_grp:
                                # last img-ch: per-group output DMA
                                nc.sync.dma_start(
                                    y_d[img, ch, :, :, 2 * g:2 * g + 2, :],
                                    ot[:, :, 2 * g:2 * g + 2, :])
                    if not (img == PER - 1 and ch == N_CH - 1):
                        nc.sync.dma_start(y_d[img, ch], ot[:])

    nc.compile()
    return nc


def _get_program():
    if "nc" not in _prog_cache:
        _prog_cache["nc"] = _build_program()
    return _prog_cache["nc"]


def _prep_inputs(x, weight, bias):
    x = np.ascontiguousarray(np.asarray(x, dtype=np.float32))
    weight = np.ascontiguousarray(np.asarray(weight, dtype=np.float32))
    bias = np.ascontiguousarray(np.asarray(bias, dtype=np.float32))

    x_pad = np.zeros((N, C_IN, HP, WP), dtype=ml_dtypes.bfloat16)
    x_pad[:, :, 1:1 + H, 1:1 + W] = x.astype(ml_dtypes.bfloat16)
    # even/odd column split: [n, ci, HP, 2, 29]
    x_pad = np.ascontiguousarray(
        x_pad.reshape(N, C_IN, HP, 29, 2).transpose(0, 1, 2, 4, 3))

    # U[xi][co,ci,kh] = sum_kw G[xi,kw] w[co,ci,kh,kw] -> [ci, ch, xi, kh, co128]
    G = np.array([[1, 0, 0], [.5, .5, .5], [.5, -.5, .5], [0, 0, 1]],
                 dtype=np.float32)
    u = np.einsum("gk,oihk->oihg", G, weight)      # [co, ci, kh, xi]
    u_t = np.ascontiguousarray(
        u.transpose(1, 2, 3, 0)                     # [ci, kh, xi, co]
        .reshape(C_IN, KH, NXI, N_CH, 128)
        .transpose(0, 3, 2, 1, 4)                   # [ci, ch, xi, kh, co128]
        [:, :, list(XI_ORDER)]                      # xi axis in matmul order
        .astype(ml_dtypes.bfloat16)
    )
    b_t = np.ascontiguousarray(bias.reshape(N_CH, 128).T)

    in_maps = []
    for c in range(N_CORES):
        in_maps.append({
            "x": x_pad[c * PER:(c + 1) * PER],
            "u": u_t,
            "b": b_t,
        })
    return in_maps


def _run(x, weight, bias, trace=False):
    nc = _get_program()
    in_maps = _prep_inputs(x, weight, bias)
    res = run_bass_kernel_spmd(
        nc, in_maps, core_ids=list(range(N_CORES)), trace=trace,
    )
    # y arrives parity-split bf16 [PER, N_CH, 128, 2, 4, FD];
    # re-interleave W and cast to f32.
    parts = [
        np.asarray(res.results[c]["y"])
        .reshape(PER, C_OUT, 2, H, NT).astype(np.float32)
        .transpose(0, 1, 3, 4, 2).reshape(PER, C_OUT, H, W)
        for c in range(N_CORES)
    ]
    y = np.concatenate(parts, axis=0)
    return y, res


def kernel(x, weight, bias):
    y, _ = _run(x, weight, bias, trace=False)
    return y


# revision 46
# speedup vs baseline: 1.0353x; 1.0353x over previous
"""Conv2d 3x3 same-padding, NCHW, on 8 TRN2 NeuronCores (data-parallel).

Problem: x[32,128,56,56] f32, weight[256,128,3,3] OIHW, bias[256] ->
y[32,256,56,56].  Batch is sharded 4 images/core; weight+bias replicated.

Per-core Winograd F(2,3) along W (bf16 matmuls, fp32 PSUM):
  - W=56 output cols -> 28 tiles of 2.  For tile j the 4 input taps are
    padded cols 2j..2j+3.  Input transform (W only, per row):
      v0 = d0-d2, v1 = d1+d2, v2 = d2-d1, v3 = d1-d3
    all computed on DVE (fastest engine for bf16 tensor_tensor), one
    image ahead, as 8 half-image sub-ops spread over the groups.
  - weights are host-transformed: U[xi][co,ci,kh] = sum_kw G[xi,kw] w[..kw],
    G = [[1,0,0],[.5,.5,.5],[.5,-.5,.5],[0,0,1]], laid out as
    [ci, ch, xi, kh, co128] bf16 so each (ch,xi,kh) is a [128,128] lhsT.
  - PE per (img, ch-half, 28-row GROUP = 2 chunks of 14 rows): per xi one
    PSUM tile [128, 2, 512] f32 (2 banks); chunk c -> ps[:, c, 0:392].
    24 matmuls of 392 cols per group; all 8 PSUM banks in use; group
    pipelining relies on per-xi-tile WAR (evictions free banks in xi
    order 1,2,0,3 matching the matmul order).
  - combine per group (y0 = m0+m1+m2+b, y1 = m1-m2-m3+b) with wide
    [2,392] strided PSUM reads, engine-balanced by measured rates
    (ACT (172+FD)/1.2, DVE (120+FD)/0.96 psum / (58+FD/2)/0.96 bf16,
    GPS ~2.1ns/elem, and GPS shares its SBUF port with DVE so it gets
    exactly one op per group):
      ACT: e1 = Id(m1+b), e2 = Copy(m2)            (bf16 out)
      DVE: q = m0+e1, y0 = q+e2, y1 = d-m3         -> ot bf16
      GPS: d = e1-e2
    q = m0+e1 frees m0's banks with only the e1 dependency, keeping the
    next-group WAR chain short (~3us slack).
  - DMA: all input DMAs go on the sync queue, whose per-queue emission
    order is the ring service order (the 16 DMA engines round-robin
    BETWEEN queues, so a big transfer on another queue would starve the
    startup-critical x0/u0 transfers; the Tile scheduler also hoists
    dependency-free DMA issues to t~0 regardless of program position).
    Priority order: x0 rows 0..15, u[ch0] first half (xi 1,2 in matmul
    order thanks to the host-side xi permute), x0 rows 16..29, u[ch0]
    second half, then the rest after the first group's matmuls are
    enqueued; xp for image i+1 is issued a full image ahead.
  - output is bf16 parity-split [128, 2, 4, 392]; host re-interleaves W
    and casts to f32.  One DMA per (img, ch-half); the last img-ch is
    drained per chunk so only ~1 chunk's combine+DMA trails the last mm.
  - image 0 x is DMA'd in 4 disjoint row-ranges so the PE starts after
    ~230 KB lands; warm-up matmuls trip the PE HAM clock-gate during the
    startup window.
"""

import ml_dtypes
import numpy as np

import concourse.bacc as bacc
import concourse.mybir as mybir
import concourse.tile as tile
from concourse.bass_utils import run_bass_kernel_spmd

N_CORES = 8
N, C_IN, H, W = 32, 128, 56, 56
C_OUT, KH, KW = 256, 3, 3
PER = N // N_CORES          # images per core
HP, WP = H + 2, W + 2       # zero-padded image dims
NT = W // 2                 # 28 winograd tiles along W
NXI = 4                     # winograd terms
RPC = 14                    # output rows per chunk
FD = RPC * NT               # 392 matmul cols per chunk
N_GRP = 2                   # 2-chunk groups per img-ch (28 rows each)
N_CH = C_OUT // 128         # output-channel halves
WARMUP_MMS = 3
# matmul group order (evictions start after m1, m2); the host stores U's
# xi axis permuted into this order so u[ch0] splits into two contiguous
# priority DMAs (first half = xi 1,2 gates the first matmuls)
XI_ORDER = (1, 2, 0, 3)
# image-0 row ranges (disjoint, cover 0..57)
R0 = [(0, 16), (16, 30), (30, 44), (44, 58)]
# half-image row splits for the look-ahead V transform
VR = [(0, 29), (29, 58)]

f32 = mybir.dt.float32
bf16 = mybir.dt.bfloat16
ADD = mybir.AluOpType.add
SUB = mybir.AluOpType.subtract
COPY = mybir.ActivationFunctionType.Copy
IDENT = mybir.ActivationFunctionType.Identity

_prog_cache = {}


def _build_program():
    nc = bacc.Bacc("TRN2", target_bir_lowering=False, debug=False)
    # x is host-split into even/odd column planes [.., HP, 2, 29] and y is
    # produced parity-split bf16 [.., 2, 4, 392] (host re-interleaves after
    # the run) so every vector-engine access on chip is contiguous.
    x_d = nc.declare_dram_parameter("x", [PER, C_IN, HP, 2, 29], bf16, isOutput=False)
    u_d = nc.declare_dram_parameter("u", [C_IN, N_CH, NXI, KH, 128], bf16, isOutput=False)
    b_d = nc.declare_dram_parameter("b", [128, N_CH], f32, isOutput=False)
    y_d = nc.declare_dram_parameter("y", [PER, N_CH, 128, 2, 4, FD], bf16, isOutput=True)

    with tile.TileContext(nc) as tc:
        with (
            tc.tile_pool(name="wpool", bufs=1) as wpool,
            tc.tile_pool(name="x0pool", bufs=4) as x0pool,
            tc.tile_pool(name="xppool", bufs=2) as xppool,
            tc.tile_pool(name="vpool", bufs=2) as vpool,
            tc.tile_pool(name="spool", bufs=10) as spool,
            tc.tile_pool(name="opool", bufs=3) as opool,
            tc.tile_pool(name="pspool", bufs=4, space="PSUM") as pspool,
            tc.tile_pool(name="warm", bufs=1) as warmpool,
        ):
            # PE warm-up during the startup protocol / first DMA window.
            wu_zero = warmpool.tile([128, FD], f32, tag="wuzero")
            nc.vector.memset(wu_zero[:], 0.0)
            wu_src = warmpool.tile([128, FD], bf16, tag="wusrc")
            nc.vector.tensor_copy(wu_src[:], wu_zero[:])
            wu_ps = pspool.tile([128, 2, 512], f32, tag="ps")

            u_t = wpool.tile([C_IN, N_CH, NXI, KH, 128], bf16, tag="u")
            b_t = wpool.tile([128, N_CH], f32, tag="b")

            # image-0 row-range tiles + shared V tile
            x0c = [x0pool.tile([128, 16, 2, 29], bf16, tag="x0", name=f"x0c{k}")
                   for k in range(len(R0))]
            v0 = vpool.tile([128, NXI, HP, NT], bf16, tag="v")

            # Input DMAs all stripe across the same 16 DMA engines, and
            # emission order on one queue (sync) is the ring's service
            # order.  So: only what the FIRST matmuls need goes first --
            # u[ch0,xi=1], x0 rows 0..29 -- then the rest of u[ch0] in
            # matmul (xi) order; everything else after the first group's
            # matmuls are enqueued.
            nc.sync.dma_start(x0c[0][:, 0:R0[0][1] - R0[0][0]], x_d[0, :, R0[0][0]:R0[0][1]])
            nc.sync.dma_start(x0c[1][:, 0:R0[1][1] - R0[1][0]], x_d[0, :, R0[1][0]:R0[1][1]])
            nc.sync.dma_start(u_t[:, 0, 0:2], u_d[:, 0, 0:2])
            nc.sync.dma_start(u_t[:, 0, 2:4], u_d[:, 0, 2:4])
            nc.scalar.dma_start(b_t[:], b_d[:])

            for _ in range(WARMUP_MMS):
                nc.tensor.matmul(wu_ps[:, 0, 0:FD], wu_src[:, :128], wu_src[:],
                                 start=True, stop=True)

            def v_transform(eng, vt, xt, rows_out, rows_in, only_xi=None):
                """vt[:, xi, rows_out, :] from xt rows rows_in (same rows).
                With d_t = xpad[2j+t]: v0=d0-d2, v1=d1+d2, v2=d2-d1,
                v3=d1-d3.  x is even/odd split: d0=xe[j], d2=xe[j+1],
                d1=xo[j], d3=xo[j+1] -- all contiguous 28-slices."""
                o0, o1 = rows_out
                i0, i1 = rows_in
                xe0 = xt[:, i0:i1, 0, 0:NT]
                xe1 = xt[:, i0:i1, 0, 1:NT + 1]
                xo0 = xt[:, i0:i1, 1, 0:NT]
                xo1 = xt[:, i0:i1, 1, 1:NT + 1]
                table = [
                    (0, xe0, xe1, SUB),
                    (1, xo0, xe1, ADD),
                    (2, xe1, xo0, SUB),
                    (3, xo0, xo1, SUB),
                ]
                for xi, s0, s1, op in table:
                    if only_xi is None or xi == only_xi:
                        eng.tensor_tensor(vt[:, xi, o0:o1, :], s0, s1, op)

            # image-0 V for rows 0..29 on DVE right behind the DMAs, xi-major
            # in matmul order so the PE's next-needed plane is always first
            for xi in XI_ORDER:
                for k in (0, 1):
                    v_transform(nc.vector, v0, x0c[k], R0[k],
                                (0, R0[k][1] - R0[k][0]), only_xi=xi)

            vts = {0: v0}
            xps = [xppool.tile([128, HP, 2, 29], bf16, tag="xp",
                               name=f"xp{i}") for i in range(PER - 1)]

            for img in range(PER):
                vt = vts.pop(img)
                if img + 1 < PER:
                    xp = xps[img]
                    vnxt = vpool.tile([128, NXI, HP, NT], bf16, tag="v")
                    vts[img + 1] = vnxt
                # look-ahead V sub-ops: (xi, row-range).  The xp DMA for
                # img+1 is issued a full image ahead (except img 1's, which
                # waits for the startup backlog), so by the time these run
                # the data is resident.  img 0 skips group 0 because xp1
                # lands only mid-image.
                vops = [(xi, vr) for vr in VR for xi in range(NXI)]
                vops_per_group = ({1: 3, 2: 3, 3: 2} if img == 0
                                  else {0: 2, 1: 2, 2: 2, 3: 2})
                vop_i = 0
                for ch in range(N_CH):
                    ot = opool.tile([128, 2, 2 * N_GRP, FD], bf16, tag="ot")
                    bias = b_t[:, ch:ch + 1]
                    for g in range(N_GRP):
                        last_grp = (img == PER - 1 and ch == N_CH - 1)
                        ps = {}
                        for kpos, xi in enumerate(XI_ORDER):
                            pst = pspool.tile([128, 2, 512], f32, tag="ps")
                            ps[xi] = pst
                            for kh in range(KH):
                                for c in range(2):
                                    r0 = RPC * (2 * g + c) + kh
                                    nc.tensor.matmul(
                                        pst[:, c, 0:FD],
                                        u_t[:, ch, kpos, kh, :],
                                        vt[:, xi, r0:r0 + RPC, :],
                                        start=(kh == 0),
                                        stop=(kh == KH - 1),
                                    )
                        if img == 0 and ch == 0 and g == 0:
                            # deferred input DMAs, still priority-ordered on
                            # the sync queue behind u0/x0c0/x0c1
                            nc.sync.dma_start(x0c[2][:, 0:R0[2][1] - R0[2][0]],
                                              x_d[0, :, R0[2][0]:R0[2][1]])
                            nc.sync.dma_start(x0c[3][:, 0:R0[3][1] - R0[3][0]],
                                              x_d[0, :, R0[3][0]:R0[3][1]])
                            nc.sync.dma_start(u_t[:, 1], u_d[:, 1])
                            for xi in XI_ORDER:
                                for k in (2, 3):
                                    v_transform(nc.vector, v0, x0c[k], R0[k],
                                                (0, R0[k][1] - R0[k][0]),
                                                only_xi=xi)
                        if img == 0 and ch == 0 and g == 1 and PER > 1:
                            nc.sync.dma_start(xps[0][:], x_d[1])
                        if ch == 1 and g == 0 and img + 2 < PER:
                            nc.sync.dma_start(xps[img + 1][:], x_d[img + 2])
                        # look-ahead V sub-ops on DVE, FIRST in the group's
                        # DVE FIFO slot: they have no unmet deps, so they run
                        # in the PE's m1/m2 window while DVE is idle, and the
                        # image's full V is ready before the next image's
                        # first matmul.
                        if img + 1 < PER:
                            for _ in range(vops_per_group.get(2 * ch + g, 0)):
                                xi, (a, b) = vops[vop_i]
                                vop_i += 1
                                xe0 = xp[:, a:b, 0, 0:NT]
                                xe1 = xp[:, a:b, 0, 1:NT + 1]
                                xo0 = xp[:, a:b, 1, 0:NT]
                                xo1 = xp[:, a:b, 1, 1:NT + 1]
                                src0, src1, op = [
                                    (xe0, xe1, SUB),   # v0 = d0-d2
                                    (xo0, xe1, ADD),   # v1 = d1+d2
                                    (xe1, xo0, SUB),   # v2 = d2-d1
                                    (xo0, xo1, SUB),   # v3 = d1-d3
                                ][xi]
                                nc.vector.tensor_tensor(
                                    vts[img + 1][:, xi, a:b, :], src0, src1, op)
                        # combine: y0 = m0+m1+m2+b, y1 = m1-m2-m3+b over
                        # both chunks at once ([2,392] strided PSUM reads).
                        # q = m0+e1 on DVE frees m0's banks with only the
                        # e1 dependency (not e2), so the next-group WAR
                        # chain is short; d on GPSIMD (its consumer y1 has
                        # slack, and one op per group keeps DVE/GPS
                        # SBUF-port contention low).
                        m0, m1, m2, m3 = (ps[i][:, :, 0:FD] for i in range(4))
                        e1 = spool.tile([128, 2, FD], bf16, tag="e1")
                        e2 = spool.tile([128, 2, FD], bf16, tag="e2")
                        qt = spool.tile([128, 2, FD], bf16, tag="q")
                        dt = spool.tile([128, 2, FD], bf16, tag="d")
                        nc.scalar.activation(e1[:], m1, IDENT, bias=bias)
                        nc.scalar.activation(e2[:], m2, COPY)
                        nc.vector.tensor_tensor(qt[:], m0, e1[:], ADD)
                        nc.vector.tensor_tensor(
                            ot[:, 0, 2 * g:2 * g + 2, :], qt[:], e2[:], ADD)
                        if last_grp and g == N_GRP - 1:
                            # final drain: per-chunk y-ops and DMAs on DVE so
                            # only ~1 chunk's combine+DMA trails the last mm
                            nc.vector.tensor_tensor(dt[:], e1[:], e2[:], SUB)
                            for c in range(2):
                                nc.vector.tensor_tensor(
                                    ot[:, 1, 2 * g + c, :], dt[:, c],
                                    ps[3][:, c, 0:FD], SUB)
                                nc.sync.dma_start(
                                    y_d[img, ch, :, :, 2 * g + c, :],
                                    ot[:, :, 2 * g + c, :])
                        else:
                            nc.gpsimd.tensor_tensor(dt[:], e1[:], e2[:], SUB)
                            nc.vector.tensor_tensor(
                                ot[:, 1, 2 * g:2 * g + 2, :], dt[:], m3, SUB)
                            if last_grp:
                                # last img-ch: per-group output DMA
                                nc.sync.dma_start(
                                    y_d[img, ch, :, :, 2 * g:2 * g + 2, :],
                                    ot[:, :, 2 * g:2 * g + 2, :])
                    if not (img == PER - 1 and ch == N_CH - 1):
                        nc.sync.dma_start(y_d[img, ch], ot[:])

    nc.compile()
    return nc


def _get_program():
    if "nc" not in _prog_cache:
        _prog_cache["nc"] = _build_program()
    return _prog_cache["nc"]


def _prep_inputs(x, weight, bias):
    x = np.ascontiguousarray(np.asarray(x, dtype=np.float32))
    weight = np.ascontiguousarray(np.asarray(weight, dtype=np.float32))
    bias = np.ascontiguousarray(np.asarray(bias, dtype=np.float32))

    x_pad = np.zeros((N, C_IN, HP, WP), dtype=ml_dtypes.bfloat16)
    x_pad[:, :, 1:1 + H, 1:1 + W] = x.astype(ml_dtypes.bfloat16)
    # even/odd column split: [n, ci, HP, 2, 29]
    x_pad = np.ascontiguousarray(
        x_pad.reshape(N, C_IN, HP, 29, 2).transpose(0, 1, 2, 4, 3))

    # U[xi][co,ci,kh] = sum_kw G[xi,kw] w[co,ci,kh,kw] -> [ci, ch, xi, kh, co128]
    G = np.array([[1, 0, 0], [.5, .5, .5], [.5, -.5, .5], [0, 0, 1]],
                 dtype=np.float32)
    u = np.einsum("gk,oihk->oihg", G, weight)      # [co, ci, kh, xi]
    u_t = np.ascontiguousarray(
        u.transpose(1, 2, 3, 0)                     # [ci, kh, xi, co]
        .reshape(C_IN, KH, NXI, N_CH, 128)
        .transpose(0, 3, 2, 1, 4)                   # [ci, ch, xi, kh, co128]
        [:, :, list(XI_ORDER)]                      # xi axis in matmul order
        .astype(ml_dtypes.bfloat16)
    )
    b_t = np.ascontiguousarray(bias.reshape(N_CH, 128).T)

    in_maps = []
    for c in range(N_CORES):
        in_maps.append({
            "x": x_pad[c * PER:(c + 1) * PER],
            "u": u_t,
            "b": b_t,
        })
    return in_maps


def _run(x, weight, bias, trace=False):
    nc = _get_program()
    in_maps = _prep_inputs(x, weight, bias)
    res = run_bass_kernel_spmd(
        nc, in_maps, core_ids=list(range(N_CORES)), trace=trace,
    )
    # y arrives parity-split bf16 [PER, N_CH, 128, 2, 4, FD];
    # re-interleave W and cast to f32.
    parts = [
        np.asarray(res.results[c]["y"])
        .reshape(PER, C_OUT, 2, H, NT).astype(np.float32)
        .transpose(0, 1, 3, 4, 2).reshape(PER, C_OUT, H, W)
        for c in range(N_CORES)
    ]
    y = np.concatenate(parts, axis=0)
    return y, res


def kernel(x, weight, bias):
    y, _ = _run(x, weight, bias, trace=False)
    return y


# revision 47
# speedup vs baseline: 1.0538x; 1.0179x over previous
"""Conv2d 3x3 same-padding, NCHW, on 8 TRN2 NeuronCores (data-parallel).

Problem: x[32,128,56,56] f32, weight[256,128,3,3] OIHW, bias[256] ->
y[32,256,56,56].  Batch is sharded 4 images/core; weight+bias replicated.

Per-core Winograd F(2,3) along W (bf16 matmuls, fp32 PSUM):
  - W=56 output cols -> 28 tiles of 2.  For tile j the 4 input taps are
    padded cols 2j..2j+3.  Input transform (W only, per row):
      v0 = d0-d2, v1 = d1+d2, v2 = d2-d1, v3 = d1-d3
    all computed on DVE (fastest engine for bf16 tensor_tensor), one
    image ahead, as 8 half-image sub-ops spread over the groups.
  - weights are host-transformed: U[xi][co,ci,kh] = sum_kw G[xi,kw] w[..kw],
    G = [[1,0,0],[.5,.5,.5],[.5,-.5,.5],[0,0,1]], laid out as
    [ci, ch, xi, kh, co128] bf16 so each (ch,xi,kh) is a [128,128] lhsT.
  - PE per (img, ch-half, 28-row GROUP = 2 chunks of 14 rows): per xi one
    PSUM tile [128, 2, 512] f32 (2 banks); chunk c -> ps[:, c, 0:392].
    24 matmuls of 392 cols per group; all 8 PSUM banks in use; group
    pipelining relies on per-xi-tile WAR (evictions free banks in xi
    order 1,2,0,3 matching the matmul order).
  - combine per group (y0 = m0+m1+m2+b, y1 = m1-m2-m3+b) with wide
    [2,392] strided PSUM reads, engine-balanced by measured rates
    (ACT (172+FD)/1.2, DVE (120+FD)/0.96 psum / (58+FD/2)/0.96 bf16,
    GPS ~2.1ns/elem, and GPS shares its SBUF port with DVE so it gets
    exactly one op per group):
      ACT: e1 = Id(m1+b), e2 = Copy(m2)            (bf16 out)
      DVE: q = m0+e1, y0 = q+e2, y1 = d-m3         -> ot bf16
      GPS: d = e1-e2
    q = m0+e1 frees m0's banks with only the e1 dependency, keeping the
    next-group WAR chain short (~3us slack).
  - DMA: all input DMAs go on the sync queue, whose per-queue emission
    order is the ring service order (the 16 DMA engines round-robin
    BETWEEN queues, so a big transfer on another queue would starve the
    startup-critical x0/u0 transfers; the Tile scheduler also hoists
    dependency-free DMA issues to t~0 regardless of program position).
    Priority order: x0 rows 0..15, u[ch0] first half (xi 1,2 in matmul
    order thanks to the host-side xi permute), x0 rows 16..29, u[ch0]
    second half, then the rest after the first group's matmuls are
    enqueued; xp for image i+1 is issued a full image ahead.
  - output is bf16 parity-split [128, 2, 4, 392]; host re-interleaves W
    and casts to f32.  One DMA per (img, ch-half); the last img-ch is
    drained per chunk so only ~1 chunk's combine+DMA trails the last mm.
  - image 0 x is DMA'd in 4 disjoint row-ranges so the PE starts after
    ~230 KB lands; warm-up matmuls trip the PE HAM clock-gate during the
    startup window.
"""

import ml_dtypes
import numpy as np

import concourse.bacc as bacc
import concourse.mybir as mybir
import concourse.tile as tile
from concourse.bass_utils import run_bass_kernel_spmd

N_CORES = 8
N, C_IN, H, W = 32, 128, 56, 56
C_OUT, KH, KW = 256, 3, 3
PER = N // N_CORES          # images per core
HP, WP = H + 2, W + 2       # zero-padded image dims
NT = W // 2                 # 28 winograd tiles along W
NXI = 4                     # winograd terms
RPC = 14                    # output rows per chunk
FD = RPC * NT               # 392 matmul cols per chunk
N_GRP = 2                   # 2-chunk groups per img-ch (28 rows each)
N_CH = C_OUT // 128         # output-channel halves
WARMUP_MMS = 3
# matmul group order (evictions start after m1, m2); the host stores U's
# xi axis permuted into this order so u[ch0] splits into two contiguous
# priority DMAs (first half = xi 1,2 gates the first matmuls)
XI_ORDER = (1, 2, 0, 3)
# image-0 row ranges (disjoint, cover 0..57)
R0 = [(0, 16), (16, 30), (30, 44), (44, 58)]
# half-image row splits for the look-ahead V transform
VR = [(0, 29), (29, 58)]

f32 = mybir.dt.float32
bf16 = mybir.dt.bfloat16
ADD = mybir.AluOpType.add
SUB = mybir.AluOpType.subtract
COPY = mybir.ActivationFunctionType.Copy
IDENT = mybir.ActivationFunctionType.Identity

_prog_cache = {}


def _build_program():
    nc = bacc.Bacc("TRN2", target_bir_lowering=False, debug=False)
    # x is host-split into even/odd column planes [.., HP, 2, 29] and y is
    # produced parity-split bf16 [.., 2, 4, 392] (host re-interleaves after
    # the run) so every vector-engine access on chip is contiguous.
    x_d = nc.declare_dram_parameter("x", [PER, C_IN, HP, 2, 29], bf16, isOutput=False)
    u_d = nc.declare_dram_parameter("u", [C_IN, N_CH, NXI, KH, 128], bf16, isOutput=False)
    b_d = nc.declare_dram_parameter("b", [128, N_CH], f32, isOutput=False)
    y_d = nc.declare_dram_parameter("y", [PER, N_CH, 128, 2, 4, FD], bf16, isOutput=True)

    with tile.TileContext(nc) as tc:
        with (
            tc.tile_pool(name="wpool", bufs=1) as wpool,
            tc.tile_pool(name="x0pool", bufs=4) as x0pool,
            tc.tile_pool(name="xppool", bufs=2) as xppool,
            tc.tile_pool(name="vpool", bufs=2) as vpool,
            tc.tile_pool(name="spool", bufs=10) as spool,
            tc.tile_pool(name="opool", bufs=3) as opool,
            tc.tile_pool(name="pspool", bufs=4, space="PSUM") as pspool,
            tc.tile_pool(name="warm", bufs=1) as warmpool,
        ):
            # PE warm-up during the startup protocol / first DMA window.
            wu_zero = warmpool.tile([128, FD], f32, tag="wuzero")
            nc.vector.memset(wu_zero[:], 0.0)
            wu_src = warmpool.tile([128, FD], bf16, tag="wusrc")
            nc.vector.tensor_copy(wu_src[:], wu_zero[:])
            wu_ps = pspool.tile([128, 2, 512], f32, tag="ps")

            u_t = wpool.tile([C_IN, N_CH, NXI, KH, 128], bf16, tag="u")
            b_t = wpool.tile([128, N_CH], f32, tag="b")

            # image-0 row-range tiles + shared V tile
            x0c = [x0pool.tile([128, 16, 2, 29], bf16, tag="x0", name=f"x0c{k}")
                   for k in range(len(R0))]
            v0 = vpool.tile([128, NXI, HP, NT], bf16, tag="v")

            # Input DMAs all stripe across the same 16 DMA engines, and
            # emission order on one queue (sync) is the ring's service
            # order.  So: only what the FIRST matmuls need goes first --
            # u[ch0,xi=1], x0 rows 0..29 -- then the rest of u[ch0] in
            # matmul (xi) order; everything else after the first group's
            # matmuls are enqueued.
            nc.sync.dma_start(x0c[0][:, 0:R0[0][1] - R0[0][0]], x_d[0, :, R0[0][0]:R0[0][1]])
            nc.sync.dma_start(u_t[:, 0, 0:2], u_d[:, 0, 0:2])
            nc.sync.dma_start(x0c[1][:, 0:R0[1][1] - R0[1][0]], x_d[0, :, R0[1][0]:R0[1][1]])
            nc.sync.dma_start(u_t[:, 0, 2:4], u_d[:, 0, 2:4])
            nc.scalar.dma_start(b_t[:], b_d[:])

            for _ in range(WARMUP_MMS):
                nc.tensor.matmul(wu_ps[:, 0, 0:FD], wu_src[:, :128], wu_src[:],
                                 start=True, stop=True)

            def v_transform(eng, vt, xt, rows_out, rows_in, only_xi=None):
                """vt[:, xi, rows_out, :] from xt rows rows_in (same rows).
                With d_t = xpad[2j+t]: v0=d0-d2, v1=d1+d2, v2=d2-d1,
                v3=d1-d3.  x is even/odd split: d0=xe[j], d2=xe[j+1],
                d1=xo[j], d3=xo[j+1] -- all contiguous 28-slices."""
                o0, o1 = rows_out
                i0, i1 = rows_in
                xe0 = xt[:, i0:i1, 0, 0:NT]
                xe1 = xt[:, i0:i1, 0, 1:NT + 1]
                xo0 = xt[:, i0:i1, 1, 0:NT]
                xo1 = xt[:, i0:i1, 1, 1:NT + 1]
                table = [
                    (0, xe0, xe1, SUB),
                    (1, xo0, xe1, ADD),
                    (2, xe1, xo0, SUB),
                    (3, xo0, xo1, SUB),
                ]
                for xi, s0, s1, op in table:
                    if only_xi is None or xi == only_xi:
                        eng.tensor_tensor(vt[:, xi, o0:o1, :], s0, s1, op)

            # image-0 V for rows 0..29 on DVE right behind the DMAs, xi-major
            # in matmul order so the PE's next-needed plane is always first
            for xi in XI_ORDER:
                for k in (0, 1):
                    v_transform(nc.vector, v0, x0c[k], R0[k],
                                (0, R0[k][1] - R0[k][0]), only_xi=xi)

            vts = {0: v0}
            xps = [xppool.tile([128, HP, 2, 29], bf16, tag="xp",
                               name=f"xp{i}") for i in range(PER - 1)]

            for img in range(PER):
                vt = vts.pop(img)
                if img + 1 < PER:
                    xp = xps[img]
                    vnxt = vpool.tile([128, NXI, HP, NT], bf16, tag="v")
                    vts[img + 1] = vnxt
                # look-ahead V sub-ops: (xi, row-range).  The xp DMA for
                # img+1 is issued a full image ahead (except img 1's, which
                # waits for the startup backlog), so by the time these run
                # the data is resident.  img 0 skips group 0 because xp1
                # lands only mid-image.
                vops = [(xi, vr) for vr in VR for xi in range(NXI)]
                vops_per_group = ({1: 3, 2: 3, 3: 2} if img == 0
                                  else {0: 2, 1: 2, 2: 2, 3: 2})
                vop_i = 0
                for ch in range(N_CH):
                    ot = opool.tile([128, 2, 2 * N_GRP, FD], bf16, tag="ot")
                    bias = b_t[:, ch:ch + 1]
                    for g in range(N_GRP):
                        last_grp = (img == PER - 1 and ch == N_CH - 1)
                        ps = {}
                        for kpos, xi in enumerate(XI_ORDER):
                            pst = pspool.tile([128, 2, 512], f32, tag="ps")
                            ps[xi] = pst
                            for kh in range(KH):
                                for c in range(2):
                                    r0 = RPC * (2 * g + c) + kh
                                    nc.tensor.matmul(
                                        pst[:, c, 0:FD],
                                        u_t[:, ch, kpos, kh, :],
                                        vt[:, xi, r0:r0 + RPC, :],
                                        start=(kh == 0),
                                        stop=(kh == KH - 1),
                                    )
                        if img == 0 and ch == 0 and g == 0:
                            # deferred input DMAs, still priority-ordered on
                            # the sync queue behind u0/x0c0/x0c1
                            nc.sync.dma_start(x0c[2][:, 0:R0[2][1] - R0[2][0]],
                                              x_d[0, :, R0[2][0]:R0[2][1]])
                            nc.sync.dma_start(x0c[3][:, 0:R0[3][1] - R0[3][0]],
                                              x_d[0, :, R0[3][0]:R0[3][1]])
                            nc.sync.dma_start(u_t[:, 1], u_d[:, 1])
                            for xi in XI_ORDER:
                                for k in (2, 3):
                                    v_transform(nc.vector, v0, x0c[k], R0[k],
                                                (0, R0[k][1] - R0[k][0]),
                                                only_xi=xi)
                        if img == 0 and ch == 0 and g == 1 and PER > 1:
                            nc.sync.dma_start(xps[0][:], x_d[1])
                        if ch == 1 and g == 0 and img + 2 < PER:
                            nc.sync.dma_start(xps[img + 1][:], x_d[img + 2])
                        # look-ahead V sub-ops on DVE, FIRST in the group's
                        # DVE FIFO slot: they have no unmet deps, so they run
                        # in the PE's m1/m2 window while DVE is idle, and the
                        # image's full V is ready before the next image's
                        # first matmul.
                        if img + 1 < PER:
                            for _ in range(vops_per_group.get(2 * ch + g, 0)):
                                xi, (a, b) = vops[vop_i]
                                vop_i += 1
                                xe0 = xp[:, a:b, 0, 0:NT]
                                xe1 = xp[:, a:b, 0, 1:NT + 1]
                                xo0 = xp[:, a:b, 1, 0:NT]
                                xo1 = xp[:, a:b, 1, 1:NT + 1]
                                src0, src1, op = [
                                    (xe0, xe1, SUB),   # v0 = d0-d2
                                    (xo0, xe1, ADD),   # v1 = d1+d2
                                    (xe1, xo0, SUB),   # v2 = d2-d1
                                    (xo0, xo1, SUB),   # v3 = d1-d3
                                ][xi]
                                nc.vector.tensor_tensor(
                                    vts[img + 1][:, xi, a:b, :], src0, src1, op)
                        # combine: y0 = m0+m1+m2+b, y1 = m1-m2-m3+b over
                        # both chunks at once ([2,392] strided PSUM reads).
                        # q = m0+e1 on DVE frees m0's banks with only the
                        # e1 dependency (not e2), so the next-group WAR
                        # chain is short; d on GPSIMD (its consumer y1 has
                        # slack, and one op per group keeps DVE/GPS
                        # SBUF-port contention low).
                        m0, m1, m2, m3 = (ps[i][:, :, 0:FD] for i in range(4))
                        e1 = spool.tile([128, 2, FD], bf16, tag="e1")
                        e2 = spool.tile([128, 2, FD], bf16, tag="e2")
                        qt = spool.tile([128, 2, FD], bf16, tag="q")
                        dt = spool.tile([128, 2, FD], bf16, tag="d")
                        nc.scalar.activation(e1[:], m1, IDENT, bias=bias)
                        nc.scalar.activation(e2[:], m2, COPY)
                        nc.vector.tensor_tensor(qt[:], m0, e1[:], ADD)
                        nc.vector.tensor_tensor(
                            ot[:, 0, 2 * g:2 * g + 2, :], qt[:], e2[:], ADD)
                        if last_grp and g == N_GRP - 1:
                            # final drain: per-chunk y-ops and DMAs on DVE so
                            # only ~1 chunk's combine+DMA trails the last mm
                            nc.vector.tensor_tensor(dt[:], e1[:], e2[:], SUB)
                            for c in range(2):
                                nc.vector.tensor_tensor(
                                    ot[:, 1, 2 * g + c, :], dt[:, c],
                                    ps[3][:, c, 0:FD], SUB)
                                nc.sync.dma_start(
                                    y_d[img, ch, :, :, 2 * g + c, :],
                                    ot[:, :, 2 * g + c, :])
                        else:
                            nc.gpsimd.tensor_tensor(dt[:], e1[:], e2[:], SUB)
                            nc.vector.tensor_tensor(
                                ot[:, 1, 2 * g:2 * g + 2, :], dt[:], m3, SUB)
                            if last_grp:
                                # last img-ch: per-group output DMA
                                nc.sync.dma_start(
                                    y_d[img, ch, :, :, 2 * g:2 * g + 2, :],
                                    ot[:, :, 2 * g:2 * g + 2, :])
                    if not (img == PER - 1 and ch == N_CH - 1):
                        nc.sync.dma_start(y_d[img, ch], ot[:])

    nc.compile()
    return nc


def _get_program():
    if "nc" not in _prog_cache:
        _prog_cache["nc"] = _build_program()
    return _prog_cache["nc"]


def _prep_inputs(x, weight, bias):
    x = np.ascontiguousarray(np.asarray(x, dtype=np.float32))
    weight = np.ascontiguousarray(np.asarray(weight, dtype=np.float32))
    bias = np.ascontiguousarray(np.asarray(bias, dtype=np.float32))

    x_pad = np.zeros((N, C_IN, HP, WP), dtype=ml_dtypes.bfloat16)
    x_pad[:, :, 1:1 + H, 1:1 + W] = x.astype(ml_dtypes.bfloat16)
    # even/odd column split: [n, ci, HP, 2, 29]
    x_pad = np.ascontiguousarray(
        x_pad.reshape(N, C_IN, HP, 29, 2).transpose(0, 1, 2, 4, 3))

    # U[xi][co,ci,kh] = sum_kw G[xi,kw] w[co,ci,kh,kw] -> [ci, ch, xi, kh, co128]
    G = np.array([[1, 0, 0], [.5, .5, .5], [.5, -.5, .5], [0, 0, 1]],
                 dtype=np.float32)
    u = np.einsum("gk,oihk->oihg", G, weight)      # [co, ci, kh, xi]
    u_t = np.ascontiguousarray(
        u.transpose(1, 2, 3, 0)                     # [ci, kh, xi, co]
        .reshape(C_IN, KH, NXI, N_CH, 128)
        .transpose(0, 3, 2, 1, 4)                   # [ci, ch, xi, kh, co128]
        [:, :, list(XI_ORDER)]                      # xi axis in matmul order
        .astype(ml_dtypes.bfloat16)
    )
    b_t = np.ascontiguousarray(bias.reshape(N_CH, 128).T)

    in_maps = []
    for c in range(N_CORES):
        in_maps.append({
            "x": x_pad[c * PER:(c + 1) * PER],
            "u": u_t,
            "b": b_t,
        })
    return in_maps


def _run(x, weight, bias, trace=False):
    nc = _get_program()
    in_maps = _prep_inputs(x, weight, bias)
    res = run_bass_kernel_spmd(
        nc, in_maps, core_ids=list(range(N_CORES)), trace=trace,
    )
    # y arrives parity-split bf16 [PER, N_CH, 128, 2, 4, FD];
    # re-interleave W and cast to f32.
    parts = [
        np.asarray(res.results[c]["y"])
        .reshape(PER, C_OUT, 2, H, NT).astype(np.float32)
        .transpose(0, 1, 3, 4, 2).reshape(PER, C_OUT, H, W)
        for c in range(N_CORES)
    ]
    y = np.concatenate(parts, axis=0)
    return y, res


def kernel(x, weight, bias):
    y, _ = _run(x, weight, bias, trace=False)
    return y
